# revision 1
# baseline (speedup 1.0000x reference)
"""BiLSTM-CRF Trainium2 Bass kernel.

Data-parallel over batch: 64 sequences -> 8 NeuronCores x 8 sequences.
Per core, one straight-line Bass/Tile program does:
  1. embedding gather (dma_gather from a padded DRAM copy of the table)
  2. input projections xg = W_ih @ x + b (blocked, PE matmuls)
  3. the two LSTM recurrences (H=128 on partitions, fp32 matmuls)
  4. emissions em = fc_w @ [h_f; h_b] + fc_b
  5. Viterbi forward pass producing per-step argmax history + final scores
The (cheap, O(S*B)) backtrace runs on host from the downloaded history.
"""

import os
import sys

import numpy as np

import concourse.bass as bass
import concourse.mybir as mybir
from concourse.tile import TileContext
from concourse.bass_utils import run_bass_kernel_spmd

F32 = mybir.dt.float32
U16 = mybir.dt.uint16
I16 = mybir.dt.int16
AF = mybir.ActivationFunctionType

V, E, EP = 32000, 100, 128      # vocab, embed dim, padded embed dim
HD, T = 256, 17                 # bilstm hidden (total), tagset
B, S = 64, 1024
H = HD // 2                     # per-direction hidden (128)
NC = 8                          # cores
BL = B // NC                    # local batch (8)
BLK = 64                        # recurrence block (steps) for xg staging

# This walrus build rejects >1 sync-wait per instruction; hoist extras
# onto same-engine NoOps (engines run their bb instructions in order).
def _split_multi_waits(nc):
    ctr = [0]
    for fn in nc.m.functions:
        for bb in fn.blocks:
            out = []
            changed = False
            for inst in bb.instructions:
                si = inst.sync_info
                waits = list(si.on_wait) if si is not None and si.on_wait else []
                if len(waits) > 1:
                    si.on_wait = waits[:1]
                    for w in waits[1:]:
                        ctr[0] += 1
                        out.append(mybir.InstNoOp(
                            name=f"I-waitfix-{ctr[0]}", ins=[], outs=[],
                            engine=inst.engine,
                            sync_info=mybir.SyncInfo(on_wait=[w], on_update=[]),
                        ))
                    changed = True
                out.append(inst)
            if changed:
                bb.instructions = out


def _build(s_len=S):
    SB = s_len * BL                       # tokens per core
    NCH = SB // 128                       # gather chunks
    nc = bass.Bass()

    xin = nc.dram_tensor("xin", [128, NCH, EP], F32, kind="ExternalInput")
    wih = nc.dram_tensor("wih", [2, 4, E + 1, H], F32, kind="ExternalInput")
    whh = nc.dram_tensor("whh", [2, 4, H, H], F32, kind="ExternalInput")
    fcw = nc.dram_tensor("fcw", [2, H, T], F32, kind="ExternalInput")
    fcb = nc.dram_tensor("fcb", [128, 1], F32, kind="ExternalInput")
    vlh = nc.dram_tensor("vlh", [128, 32], F32, kind="ExternalInput")
    scri = nc.dram_tensor("scri", [128, 32], F32, kind="ExternalInput")
    strep = nc.dram_tensor("strep", [128, 1], F32, kind="ExternalInput")
    enrep = nc.dram_tensor("enrep", [128, 1], F32, kind="ExternalInput")
    ident = nc.dram_tensor("ident", [128, 128], F32, kind="ExternalInput")

    hist_o = nc.dram_tensor("hist_o", [2, 128, 8 * s_len], U16, kind="ExternalOutput")
    scf_o = nc.dram_tensor("scf_o", [2, 128, 1], F32, kind="ExternalOutput")

    with TileContext(nc) as tc:
        import contextlib
        es = contextlib.ExitStack()
        with es:
            cpool = es.enter_context(tc.tile_pool(name="consts", bufs=1))
            wih_sb = cpool.tile([E + 1, 2, 4, H], F32, tag="wih")
            for d in range(2):
                for g in range(4):
                    nc.sync.dma_start(out=wih_sb[:, d, g, :], in_=wih[d, g, :, :])
            whh_sb = cpool.tile([H, 2, 4, H], F32, tag="whh")
            for d in range(2):
                for g in range(4):
                    nc.sync.dma_start(out=whh_sb[:, d, g, :], in_=whh[d, g, :, :])
            fcw_sb = cpool.tile([H, 2, T], F32, tag="fcw")
            for d in range(2):
                nc.sync.dma_start(out=fcw_sb[:, d, :], in_=fcw[d, :, :])
            fcb_sb = cpool.tile([128, 1], F32, tag="fcb")
            nc.sync.dma_start(out=fcb_sb[:], in_=fcb[:, :])
            vlh_sb = cpool.tile([128, 32], F32, tag="vlh")
            nc.sync.dma_start(out=vlh_sb[:], in_=vlh[:, :])
            str_sb = cpool.tile([128, 1], F32, tag="strep")
            nc.sync.dma_start(out=str_sb[:], in_=strep[:, :])
            enr_sb = cpool.tile([128, 1], F32, tag="enrep")
            nc.sync.dma_start(out=enr_sb[:], in_=enrep[:, :])
            id_sb = cpool.tile([128, 128], F32, tag="ident")
            nc.sync.dma_start(out=id_sb[:], in_=ident[:, :])

            # persistent big buffers
            xT = cpool.tile([128, SB], F32, tag="xT")          # [E(+pad) , tok]
            h_f = cpool.tile([128, SB], F32, tag="hf")
            h_b = cpool.tile([128, SB], F32, tag="hb")
            em_sb = [cpool.tile([128, s_len], F32, tag=f"em{c}", name=f"em{c}") for c in range(2)]
            hist_sb = [cpool.tile([128, 8 * s_len], U16, tag=f"hist{c}", name=f"hist{c}") for c in range(2)]

            # ---- phase 1: embedding gather + transpose ----
            with tc.tile_pool(name="gath", bufs=1) as gpool, \
                 tc.tile_pool(name="pst", bufs=4, space="PSUM") as pst:
                xg_t = gpool.tile([128, NCH, EP], F32, tag="xgath")
                nc.sync.dma_start(out=xg_t[:, :, :], in_=xin[:, :, :])
                for ch in range(NCH):
                    ps = pst.tile([128, 128], F32, tag="tps")
                    nc.tensor.transpose(ps[:], xg_t[:, ch, :], id_sb[:])
                    if ch % 2 == 0:
                        nc.scalar.copy(xT[:, ch * 128:(ch + 1) * 128], ps[:])
                    else:
                        nc.vector.tensor_copy(xT[:, ch * 128:(ch + 1) * 128], ps[:])
            # xT row E is all-ones (bias row for the K=E+1 matmuls): baked
            # into the padded embedding table column E on the host.

            # ---- phases 2+3: blocked xg precompute + recurrence ----
            z8 = cpool.tile([128, BL], F32, tag="z8")
            nc.vector.memset(z8[:], 0.0)
            c_pp = [cpool.tile([128, 2, BL], F32, tag=f"c{i}", name=f"c{i}") for i in range(2)]
            nc.vector.memset(c_pp[1][:], 0.0)  # c_prev for t=0

            with tc.tile_pool(name="xg", bufs=2) as xgp, \
                 tc.tile_pool(name="psxg", bufs=2, space="PSUM") as psxg, \
                 tc.tile_pool(name="psg", bufs=2, space="PSUM") as psg, \
                 tc.tile_pool(name="gsb", bufs=2) as gsbp, \
                 tc.tile_pool(name="small", bufs=3) as smp:
                n_blk = s_len // BLK
                x3 = xT[0:E + 1, :].rearrange("p (t b) -> p t b", b=BL)
                for blk in range(n_blk):
                    xg = xgp.tile([128, 2, 4, BLK * BL], F32, tag="xg")
                    for d in range(2):
                        for g in range(4):
                            ps = psxg.tile([128, BLK * BL], F32, tag="psxg")
                            if d == 0:
                                rhs = x3[:, blk * BLK:(blk + 1) * BLK, :]
                                out_ap = xg[:, d, g, :]
                            else:
                                # backward dir consumes x reversed in time:
                                # project the forward token block, write the
                                # xg columns time-reversed.
                                u_lo = s_len - (blk + 1) * BLK
                                rhs = x3[:, u_lo:u_lo + BLK, :]
                                out_ap = xg[:, d, g, :].rearrange(
                                    "p (t b) -> p t b", b=BL)[:, ::-1, :]
                            nc.tensor.matmul(
                                ps[:], wih_sb[:, d, g, :], rhs,
                                start=True, stop=True)
                            if (d * 4 + g) % 2 == 0:
                                nc.scalar.copy(out_ap, ps[:])
                            else:
                                nc.vector.tensor_copy(out_ap, ps[:])
                    for tl in range(BLK):
                        t = blk * BLK + tl
                        gps = psg.tile([128, 2, 4, BL], F32, tag="gps")
                        k = 0
                        for d in range(2):
                            if t == 0:
                                hprev = z8[:]
                            elif d == 0:
                                hprev = h_f[:, (t - 1) * BL:t * BL]
                            else:
                                hprev = h_b[:, (s_len - t) * BL:(s_len - t + 1) * BL]
                            for g in range(4):
                                nc.tensor.matmul(
                                    gps[:, d, g, :], whh_sb[:, d, g, :], hprev,
                                    start=(k == 0), stop=(k == 7))
                                k += 1
                        gsb = gsbp.tile([128, 2, 4, BL], F32, tag="gsb")
                        nc.vector.scalar_tensor_tensor(
                            out=gsb[:], in0=gps[:], scalar=0.0,
                            in1=xg[:, :, :, tl * BL:(tl + 1) * BL],
                            op0=mybir.AluOpType.add, op1=mybir.AluOpType.add)
                        sig = smp.tile([128, 2, 3, BL], F32, tag="sig")
                        nc.scalar.activation(sig[:], gsb[:, :, 0:3, :], AF.Sigmoid)
                        tg = smp.tile([128, 2, BL], F32, tag="tg")
                        nc.scalar.activation(tg[:], gsb[:, :, 3, :], AF.Tanh)
                        cprev = c_pp[(t + 1) % 2]
                        cnew = c_pp[t % 2]
                        tmp = smp.tile([128, 2, BL], F32, tag="tmp")
                        nc.vector.tensor_mul(tmp[:], sig[:, :, 0, :], tg[:])
                        nc.vector.tensor_mul(cnew[:], sig[:, :, 1, :], cprev[:])
                        nc.vector.tensor_add(cnew[:], cnew[:], tmp[:])
                        thc = smp.tile([128, 2, BL], F32, tag="thc")
                        nc.scalar.activation(thc[:], cnew[:], AF.Tanh)
                        nc.vector.tensor_mul(
                            h_f[:, t * BL:(t + 1) * BL], sig[:, 0, 2, :], thc[:, 0, :])
                        nc.vector.tensor_mul(
                            h_b[:, (s_len - 1 - t) * BL:(s_len - t) * BL],
                            sig[:, 1, 2, :], thc[:, 1, :])

            # ---- phase 4: emissions ----
            with tc.tile_pool(name="psem", bufs=2, space="PSUM") as psem:
                TCH = min(512, s_len)
                for ch in range(2):          # viterbi chain (b 0-3 / 4-7)
                    for c5 in range(s_len // TCH):
                        ps = psem.tile([128, TCH], F32, tag="psem")
                        nc.vector.memset(ps[:], 0.0)
                        for bb in range(4):
                            b_loc = ch * 4 + bb
                            rhs_f = h_f[:, c5 * TCH * BL:(c5 + 1) * TCH * BL] \
                                .rearrange("p (t b) -> p t b", b=BL)[:, :, b_loc]
                            rhs_b = h_b[:, c5 * TCH * BL:(c5 + 1) * TCH * BL] \
                                .rearrange("p (t b) -> p t b", b=BL)[:, :, b_loc]
                            nc.tensor.matmul(
                                ps[32 * bb:32 * bb + T, :], fcw_sb[:, 0, :], rhs_f,
                                start=True, stop=False,
                                tile_position=(0, 32 * bb))
                            nc.tensor.matmul(
                                ps[32 * bb:32 * bb + T, :], fcw_sb[:, 1, :], rhs_b,
                                start=False, stop=True,
                                tile_position=(0, 32 * bb))
                        nc.scalar.activation(
                            em_sb[ch][:, c5 * TCH:(c5 + 1) * TCH], ps[:],
                            AF.Identity, bias=fcb_sb[:, 0:1])

            # ---- phase 5: viterbi forward ----
            with tc.tile_pool(name="vit", bufs=1) as vp, \
                 tc.tile_pool(name="psv", bufs=4, space="PSUM") as psv:
                scr = [vp.tile([128, 32], F32, tag=f"scr{c}", name=f"scr{c}") for c in range(2)]
                rhsr = [vp.tile([128, 32], F32, tag=f"rhsr{c}", name=f"rhsr{c}") for c in range(2)]
                ns = [vp.tile([128, T], F32, tag=f"ns{c}", name=f"ns{c}") for c in range(2)]
                for c in range(2):
                    nc.sync.dma_start(out=scr[c][:], in_=scri[:, :])
                    nc.vector.tensor_add(
                        scr[c][:, 0:1], em_sb[c][:, 0:1], str_sb[:])
                for k in range(1, s_len):
                    for c in range(2):
                        nc.vector.transpose(rhsr[c][:], scr[c][:])
                        pv = psv.tile([128, T], F32, tag=f"pv{c}")
                        for bb in range(4):
                            nc.tensor.matmul(
                                pv[32 * bb:32 * bb + 32, :],
                                vlh_sb[32 * bb:32 * bb + 25, 0:32],
                                rhsr[c][32 * bb:32 * bb + 25, 0:T],
                                start=True, stop=True,
                                tile_position=(32 * bb, 32 * bb))
                        nc.scalar.activation(
                            ns[c][:], pv[:], AF.Identity,
                            bias=em_sb[c][:, k:k + 1])
                        nc.vector.max(scr[c][:, 0:8], ns[c][:])
                        nc.vector.max_index(
                            hist_sb[c][:, 8 * k:8 * k + 8],
                            scr[c][:, 0:8], ns[c][:])
                for c in range(2):
                    scf = vp.tile([128, 1], F32, tag=f"scf{c}")
                    nc.vector.tensor_add(scf[:], scr[c][:, 0:1], enr_sb[:])
                    nc.sync.dma_start(out=scf_o[c, :, :], in_=scf[:])
                    nc.sync.dma_start(out=hist_o[c, :, :], in_=hist_sb[c][:])

    _split_multi_waits(nc)
    return nc


_NC_CACHE = {}


def _get_nc(s_len):
    if s_len not in _NC_CACHE:
        _NC_CACHE[s_len] = _build(s_len)
    return _NC_CACHE[s_len]


def _host_inputs(sentence, embed, w_ih_f, w_hh_f, b_ih_f, b_hh_f,
                 w_ih_b, w_hh_b, b_ih_b, b_hh_b, fc_w, fc_b,
                 start_trans, end_trans, trans, s_len):
    """Build the per-core input maps (weights replicated, batch sharded)."""
    ep = np.zeros((V, EP), np.float32)
    ep[:, :E] = np.asarray(embed, np.float32)
    ep[:, E] = 1.0  # ones column -> bias row of xT after transpose

    wih = np.zeros((2, 4, E + 1, H), np.float32)
    whh = np.zeros((2, 4, H, H), np.float32)
    # pytorch gate order in rows: i, f, g, o ; our slot order: i, f, o, g
    slot2pt = [0, 1, 3, 2]
    for d, (w_ih, w_hh, b_ih, b_hh) in enumerate(
            [(w_ih_f, w_hh_f, b_ih_f, b_hh_f), (w_ih_b, w_hh_b, b_ih_b, b_hh_b)]):
        w_ih = np.asarray(w_ih, np.float32)
        w_hh = np.asarray(w_hh, np.float32)
        bias = np.asarray(b_ih, np.float32) + np.asarray(b_hh, np.float32)
        for gs in range(4):
            pt = slot2pt[gs]
            rows = slice(pt * H, (pt + 1) * H)
            wih[d, gs, :E, :] = w_ih[rows, :].T
            wih[d, gs, E, :] = bias[rows]
            whh[d, gs, :, :] = w_hh[rows, :].T

    fc_w = np.asarray(fc_w, np.float32)
    fcw = np.stack([fc_w[:, :H].T.copy(), fc_w[:, H:].T.copy()])  # [2,128,17]
    fcb = np.zeros((128, 1), np.float32)
    trans = np.asarray(trans, np.float32)
    scri = np.zeros((128, 32), np.float32)
    strep = np.zeros((128, 1), np.float32)
    enrep = np.zeros((128, 1), np.float32)
    for bb in range(4):
        fcb[32 * bb:32 * bb + T, 0] = np.asarray(fc_b, np.float32)
        scri[32 * bb:32 * bb + T, 8:8 + T] = trans  # scr[32b+i, 8+k'] = trans[i,k']
        strep[32 * bb:32 * bb + T, 0] = np.asarray(start_trans, np.float32)
        enrep[32 * bb:32 * bb + T, 0] = np.asarray(end_trans, np.float32)
    vlh = np.zeros((128, 32), np.float32)
    for bb in range(4):
        vlh[32 * bb, :T] = 1.0
        vlh[32 * bb + 8:32 * bb + 8 + T, :T] = np.eye(T, dtype=np.float32)
    ident = np.eye(128, dtype=np.float32)

    sentence = np.asarray(sentence)
    base = {
        "wih": wih, "whh": whh, "fcw": fcw, "fcb": fcb,
        "vlh": vlh, "scri": scri, "strep": strep, "enrep": enrep,
        "ident": ident,
    }
    in_maps = []
    for c in range(NC):
        sl = sentence[c * BL:(c + 1) * BL, :s_len]          # [BL, s]
        toks = sl.T.reshape(-1)                             # tok = t*BL + b
        x = ep[toks]                                        # [SB, EP]
        m = dict(base)
        m["xin"] = np.ascontiguousarray(
            x.reshape(-1, 128, EP).transpose(1, 0, 2))      # [128, NCH, EP]
        in_maps.append(m)
    return in_maps


def kernel(sentence, mask, embed, w_ih_f, w_hh_f, b_ih_f, b_hh_f,
           w_ih_b, w_hh_b, b_ih_b, b_hh_b, fc_w, fc_b,
           start_trans, end_trans, trans, _s_len=None, _profile=False):
    s_len = _s_len or np.asarray(sentence).shape[1]
    nc = _get_nc(s_len)
    in_maps = _host_inputs(sentence, embed, w_ih_f, w_hh_f, b_ih_f, b_hh_f,
                           w_ih_b, w_hh_b, b_ih_b, b_hh_b, fc_w, fc_b,
                           start_trans, end_trans, trans, s_len)
    res = run_bass_kernel_spmd(nc, in_maps, core_ids=list(range(NC)),
                               trace=_profile)
    out = np.zeros((B, s_len), np.int32)
    for c in range(NC):
        r = res.results[c]
        hist = r["hist_o"].reshape(2, 4, 32, s_len, 8)[:, :, :T, :, 0]  # ch,bb,j,k
        scf = r["scf_o"].reshape(2, 4, 32)[:, :, :T]                    # ch,bb,j
        hist = hist.reshape(8, T, s_len).astype(np.int64)               # [b_loc,j,k]
        scf = scf.reshape(8, T)
        y = np.argmax(scf, axis=1)
        path = np.zeros((8, s_len), np.int64)
        path[:, s_len - 1] = y
        bi = np.arange(8)
        for k in range(s_len - 1, 0, -1):
            y = hist[bi, y, k]
            path[:, k - 1] = y
        out[c * BL:(c + 1) * BL] = path
    if _profile:
        return out, res
    return out

